# revision 36
# baseline (speedup 1.0000x reference)
"""Trainium2 Bass kernel for nn_MultiHeadCrossAttention (B=4, T=1024, E=1024, H=16).

Sharding: 8 fully independent shards (output stream s, batch b), zero
cross-core communication. Stream-1 output xo@Wout1 needs K,V from x and Q
from y; stream-2 the reverse. Core c<4 computes stream-1 batch c; core c>=4
stream-2 batch c-4.

Per-core design: one flat software-pipelined stream over 64 (head-pair m,
key-chunk jc) units. ScalarE exp is the pacing target; all other work is
interleaved so the tensor engine keeps a dense backlog (HAM stays warm):
  unit (m, jc): S^T = K^T.T @ Q^T (row-paired K=64 MMs, [128,T] PSUM,
                one LDWEIGHTS per 2 MMs); P^T = exp(S^T/8) (2 ACTs);
                O' accumulates both ic halves per weight load
                ([128,512] 1-bank PSUM tiles, ones-row gives rowsums free).
  injections (relative to unit stream):
    m=0 units carry V-projection for chunk jc just-in-time (chunks 0-3 on
    the still-empty O' slots, 4-7 on the S pool).
    (m,0): normalize m-1: rowsum rows spread to partitions 0/32/64/96 so
           reciprocal_approx_fast runs 512-wide; K=1 ones-matmul broadcast;
           DVE mul into O^T f16. Frees the four O' accumulator banks.
    (m,1)/(m,2): Q^T/K^T projections for chunk m+1 into the freed O' banks
           (time-shared; weights streamed from HBM in host-swizzled
           contiguous blocks, one LDWEIGHTS per 2 MMs).
    (m,4): O' allocators + catch-up (deferred so the slot wait lands after
           proj released the banks; all waits resolve acyclically vs the
           in-order PE queue - anything a PE instruction waits on is
           produced by instructions emitted earlier).
  tail: normalize m=7, then Z^T = Wout^T.T @ O^T per 128-row chunk (m=7
  term accumulated last so Z overlaps the flush).
PSUM budget (8 banks): S units 2x[128,1024]=4, O'/proj/V time-shared
4x[128,512]=4; bc broadcasts borrow an S slot transiently.
Host pre-transposes/groups weights and activations; re-transposes outputs.
"""

import os
import sys

sys.path.insert(0, "/opt/trn_rl_repo")

import numpy as np
import ml_dtypes
from contextlib import ExitStack

import concourse.bass as bass
import concourse.mybir as mybir
import concourse.tile as tile
from concourse import bacc
from concourse import bass_utils

B, T, E, H = 4, 1024, 1024, 16
D = E // H            # 64
NC = E // 128         # 8 chunks of 128
NIC = T // 512        # 2 free-dim chunks of 512
N_CORES = 8

F32 = mybir.dt.float32
F16 = mybir.dt.float16
EXP = mybir.ActivationFunctionType.Exp

_NC_CACHE = {}
LAST_RESULTS = {}


def _build():
    nc = bacc.Bacc("TRN2", target_bir_lowering=False, debug=False,
                   enable_asserts=False, num_devices=N_CORES)
    a_t = nc.dram_tensor("a_t", (E, T), F16, kind="ExternalInput").ap()
    b_t = nc.dram_tensor("b_t", (E, T), F16, kind="ExternalInput").ap()
    wq_t = nc.dram_tensor("wq_t", (E, E), F16, kind="ExternalInput").ap()
    wk_t = nc.dram_tensor("wk_t", (E, E), F16, kind="ExternalInput").ap()
    wv_t = nc.dram_tensor("wv_t", (E, E), F16, kind="ExternalInput").ap()
    wout_t = nc.dram_tensor("wout_t", (E, E), F16, kind="ExternalInput").ap()
    z_t = nc.dram_tensor("z_t", (E, T), F32, kind="ExternalOutput").ap()

    # wq_t/wk_t/wout_t are host-swizzled: block m of w^T is the contiguous
    # [128, 1024] slice rows m*128..(m+1)*128, laid out [p, e*128+c].
    def wblock(w, m):
        return w[m * 128:(m + 1) * 128, :]

    with tile.TileContext(nc) as tc, ExitStack() as ctx:
        persist = ctx.enter_context(tc.tile_pool(name="persist", bufs=1))
        qt = persist.tile([128, NC, T], F16, tag="qt")
        kt = persist.tile([128, NC, T], F16, tag="kt")
        v = persist.tile([128, NC, H * (D + 1)], F16, tag="v")
        at_sb = persist.tile([128, NC, T], F16, tag="at")
        bt_sb = persist.tile([128, NC, T], F16, tag="bt")
        ot = persist.tile([128, NC, T], F16, tag="ot")      # normalized O^T
        ones_t = persist.tile([128, 128], F16, tag="ones")
        nc.vector.memset(ones_t[:], 1.0)
        # tiny dummy exp so the ~2.7us ACT_TABLE_LOAD happens during the
        # prologue DMA wait instead of right before the first real exp
        tbl = persist.tile([1, 16], F16, tag="tbl")
        nc.scalar.activation(tbl[:], ones_t[0:1, 0:16], EXP, scale=0.125)

        # DMA order matters: bt + first weight chunks first so Q0/K0 start
        # early; at/wv interleaved so V-proj can chase the transfers.
        wch = ctx.enter_context(tc.tile_pool(name="wch", bufs=2))
        wq0 = wch.tile([128, NC * 128], F16, tag="w", name="wq0")
        wk0 = wch.tile([128, NC * 128], F16, tag="w", name="wk0")
        for c in range(NC):
            nc.sync.dma_start(bt_sb[:, c, :], b_t[c * 128:(c + 1) * 128, :])
        nc.sync.dma_start(wq0[:], wblock(wq_t, 0))
        nc.sync.dma_start(wk0[:], wblock(wk_t, 0))
        for c in range(NC):
            nc.sync.dma_start(at_sb[:, c, :], a_t[c * 128:(c + 1) * 128, :])

        wvp = ctx.enter_context(tc.tile_pool(name="wv", bufs=1))
        wv_sb = wvp.tile([128, NC, E], F16, tag="wv")
        for c in range(NC):
            nc.sync.dma_start(wv_sb[:, c, :], wv_t[c * 128:(c + 1) * 128, :])

        # ones column per head in V (col D within each D+1 group) -> rowsums
        for m in range(NC):
            nc.vector.memset(
                v[:, m, :].rearrange("p (h x) -> p h x", x=D + 1)[:, :, D:D + 1], 1.0)

        ptp = ctx.enter_context(tc.tile_pool(name="pt", bufs=4))
        rsp = ctx.enter_context(tc.tile_pool(name="rsp", bufs=1))
        bcp = ctx.enter_context(tc.tile_pool(name="bcp", bufs=1))
        bigp = ctx.enter_context(tc.tile_pool(name="bigp", bufs=2, space="PSUM"))
        opool = ctx.enter_context(tc.tile_pool(name="op", bufs=4, space="PSUM"))

        def proj_chunk(w_sb, act_sb, out_sb, m, on_act, use_opool=False):
            """One [128, T] projection chunk (contract over e, weight block
            loaded once, both ic-half accumulators live)."""
            if use_opool:
                ps0 = opool.tile([128, 512], F32, tag="o", name=f"pj{m}a")
                ps1 = opool.tile([128, 512], F32, tag="o", name=f"pj{m}b")
            else:
                big = bigp.tile([128, T], F32, tag="big", name=f"pj{m}")
                ps0, ps1 = big[:, 0:512], big[:, 512:1024]
            for e in range(NC):
                st_e = dict(start=(e == 0), stop=(e == NC - 1))
                nc.tensor.matmul(ps0[:, :] if use_opool else ps0,
                                 w_sb[:, bass.ts(e, 128)],
                                 act_sb[:, e, 0:512], **st_e)
                nc.tensor.matmul(ps1[:, :] if use_opool else ps1,
                                 w_sb[:, bass.ts(e, 128)],
                                 act_sb[:, e, 512:1024], **st_e)
            for ic, ps in ((0, ps0), (1, ps1)):
                dst = out_sb[:, m, bass.ts(ic, 512)]
                if on_act:
                    nc.scalar.copy(dst, ps[:, :] if use_opool else ps)
                else:
                    nc.vector.tensor_copy(dst, ps[:, :] if use_opool else ps)

        def vproj_chunk(jc):
            """V chunk jc (natural layout), copy strided into (h, D+1).

            Chunks 0-3 use the (still empty) O'-accumulator slots so they
            don't contend with the S-unit double buffer."""
            if jc < 4:
                ps0 = opool.tile([128, 512], F32, tag="o", name=f"vp{jc}a")
                ps1 = opool.tile([128, 512], F32, tag="o", name=f"vp{jc}b")
                halves = (ps0, ps1)
            else:
                big = bigp.tile([128, T], F32, tag="big", name=f"vps{jc}")
                halves = (big[:, 0:512], big[:, 512:1024])
            for e in range(NC):
                for ic in range(NIC):
                    nc.tensor.matmul(
                        halves[ic], at_sb[:, e, bass.ts(jc, 128)],
                        wv_sb[:, e, bass.ts(ic, 512)],
                        start=(e == 0), stop=(e == NC - 1))
            for ic in range(NIC):
                nc.vector.tensor_copy(
                    v[:, jc, :].rearrange(
                        "p (h x) -> p h x", x=D + 1)[:, ic * 8:(ic + 1) * 8, 0:D],
                    halves[ic].rearrange("p (h x) -> p h x", x=D))

        # HAM warm-up: dense junk matmuls on the first bt chunk while the
        # remaining input DMAs stream, so Q0/K0 run at full clock
        warm = bigp.tile([128, T], F32, tag="big", name="warm")
        for _ in range(16):
            nc.tensor.matmul(warm[:, 0:512], bt_sb[:, 0, 0:128],
                             bt_sb[:, 0, 0:512], start=True, stop=True)

        proj_chunk(wq0, bt_sb, qt, 0, on_act=False)
        proj_chunk(wk0, at_sb, kt, 0, on_act=False)

        # per-m pipeline state
        state = {}

        NORM_ROWS = {(0, 0): 0, (0, 1): 32, (1, 0): 64, (1, 1): 96}

        def normalize_pre(m):
            """DVE-only prefix: rowsums -> 1/r (f16). The four rowsum rows
            are spread to partitions 0/32/64/96 so the reciprocal runs 512
            elements wide instead of 2048 serial."""
            st_ = state[m]
            rsf = rsp.tile([128, 512], F32, tag="rsf", name=f"rsf{m}")
            rsr = rsp.tile([128, 512], F32, tag="rsr", name=f"rsr{m}")
            rsr16 = rsp.tile([128, 512], F16, tag="rsr16", name=f"rsr16_{m}")
            for (h, ic), p in NORM_ROWS.items():
                nc.vector.tensor_copy(rsf[p:p + 1, :],
                                      st_[f"acc{ic}_{h}"][64:65, :])
            nc.vector.reciprocal_approx_fast(rsr[:], rsf[:])
            with nc.allow_low_precision(reason="1/r feeds f16 bc matmul"):
                nc.vector.tensor_copy(rsr16[:], rsr[:])
            st_["rsr16"] = rsr16

        def normalize_post(m):
            """Broadcast (K=1 ones-matmul) + DVE mul into O^T f16. Frees all
            four of m's O'-accumulator PSUM tiles; its bc matmuls precede
            any PE instruction waiting on those slots."""
            st_ = state[m]
            rsr16 = st_["rsr16"]
            bc = bigp.tile([128, T], F32, tag="big", name=f"bc{m}")
            for (h, ic), p in NORM_ROWS.items():
                nc.tensor.matmul(
                    bc[h * 64:(h + 1) * 64, bass.ts(ic, 512)],
                    ones_t[p:p + 1, 0:64], rsr16[p:p + 1, :],
                    start=True, stop=True, tile_position=(p, h * 64))
            bcs = bcp.tile([128, T], F32, tag="bcs", name=f"bcs{m}")
            nc.vector.tensor_copy(bcs[:], bc[:])
            with nc.allow_low_precision(reason="O^T f16 feeds f16 out-proj"):
                for ic in range(NIC):
                    s_ic = bass.ts(ic, 512)
                    nc.vector.tensor_mul(ot[0:64, m, s_ic],
                                         st_[f"acc{ic}_0"][0:64, :], bcs[0:64, s_ic])
                    nc.vector.tensor_mul(ot[64:128, m, s_ic],
                                         st_[f"acc{ic}_1"][0:64, :],
                                         bcs[64:128, s_ic])

        # ---------------- the 64-unit stream ----------------
        for u in range(NC * NC):
            m, jc = divmod(u, NC)
            hA, hB = 2 * m, 2 * m + 1
            if jc == 0:
                state[m] = {
                    "ptA": ptp.tile([128, NC, T], F16, tag="pt", name=f"ptA{m}"),
                    "ptB": ptp.tile([128, NC, T], F16, tag="pt", name=f"ptB{m}"),
                }
            st_ = state[m]
            ptA, ptB = st_["ptA"], st_["ptB"]

            ps_s = bigp.tile([128, T], F32, tag="big", name=f"sA{u}")
            ps_sB = bigp.tile([128, T], F32, tag="big", name=f"sB{u}")
            for ic in range(NIC):
                nc.tensor.matmul(
                    ps_s[:, bass.ts(ic, 512)],
                    kt[0:64, m, bass.ts(jc, 128)],
                    qt[0:64, m, bass.ts(ic, 512)],
                    start=True, stop=True)
            for ic in range(NIC):
                nc.tensor.matmul(
                    ps_sB[:, bass.ts(ic, 512)],
                    kt[64:128, m, bass.ts(jc, 128)],
                    qt[64:128, m, bass.ts(ic, 512)],
                    start=True, stop=True, tile_position=(64, 0))
            nc.scalar.activation(ptA[:, jc, :], ps_s[:], EXP, scale=0.125)
            nc.scalar.activation(ptB[:, jc, :], ps_sB[:], EXP, scale=0.125)

            if m == 0:
                vproj_chunk(jc)   # just-in-time V for O' below

            # ---- injections (pipelined work of m-1 / m+1); emitted BEFORE
            # this unit's O' matmuls so normalize's bc matmuls precede any
            # PE instruction that waits on the slots its muls release ----
            if jc == 0 and m >= 1:
                normalize_pre(m - 1)
            elif jc == 1 and m >= 1:
                normalize_post(m - 1)
                del state[m - 1]
            elif jc == 2 and m >= 1 and m + 1 < NC:
                wq_sb = wch.tile([128, NC * 128], F16, tag="w", name=f"wq{m+1}")
                nc.sync.dma_start(wq_sb[:], wblock(wq_t, m + 1))
                proj_chunk(wq_sb, bt_sb, qt, m + 1, on_act=False, use_opool=True)
            elif jc == 3 and m >= 1 and m + 1 < NC:
                wk_sb = wch.tile([128, NC * 128], F16, tag="w", name=f"wk{m+1}")
                nc.sync.dma_start(wk_sb[:], wblock(wk_t, m + 1))
                proj_chunk(wk_sb, at_sb, kt, m + 1, on_act=False, use_opool=True)
            elif jc == 4 and m == 0:
                wq_sb = wch.tile([128, NC * 128], F16, tag="w", name="wq1")
                nc.sync.dma_start(wq_sb[:], wblock(wq_t, 1))
                proj_chunk(wq_sb, bt_sb, qt, 1, on_act=False)
            elif jc == 6 and m == 0:
                wk_sb = wch.tile([128, NC * 128], F16, tag="w", name="wk1")
                nc.sync.dma_start(wk_sb[:], wblock(wk_t, 1))
                proj_chunk(wk_sb, at_sb, kt, 1, on_act=False)

            # O' accumulation (both ic halves per weight load). Start
            # deferred to jc==4 so the slot wait lands after proj(m+1) has
            # released the time-shared accumulator slots.
            def o_mms(j2, first):
                stf = dict(start=first, stop=(j2 == NC - 1))
                for ic in range(NIC):
                    nc.tensor.matmul(st_[f"acc{ic}_0"][0:65, :],
                                     v[:, j2, bass.ts(hA, D + 1)],
                                     ptA[:, j2, bass.ts(ic, 512)], **stf)
                for ic in range(NIC):
                    nc.tensor.matmul(st_[f"acc{ic}_1"][0:65, :],
                                     v[:, j2, bass.ts(hB, D + 1)],
                                     ptB[:, j2, bass.ts(ic, 512)], **stf)

            if jc == 4:
                for ic in range(NIC):
                    for h, nm_ in ((0, "A"), (1, "B")):
                        st_[f"acc{ic}_{h}"] = opool.tile(
                            [128, 512], F32, tag="o", name=f"o{ic}{nm_}{m}")
                for j2 in range(3):
                    o_mms(j2, first=(j2 == 0))
            elif jc == 5:
                for j2 in (3, 4, 5):
                    o_mms(j2, first=False)
            elif jc > 5:
                o_mms(jc, first=False)

        # ---------------- flush m=7 + out-projection ----------------
        # Z chunk partials (m=0..6 terms) run while the m=7 normalize chain
        # is on DVE; each finish adds the m=7 term once ot[:,7] lands.
        with tc.tile_pool(name="woch", bufs=3) as wochp, \
             tc.tile_pool(name="zsb", bufs=2) as zsbp:
            zparts = {}

            def z_partial(cc):
                wo_sb = wochp.tile([128, NC * 128], F16, tag="wo", name=f"wo{cc}")
                nc.sync.dma_start(wo_sb[:], wblock(wout_t, cc))
                ps = bigp.tile([128, T], F32, tag="big", name=f"z{cc}")
                for i in range(NC - 1):
                    for ic in range(NIC):
                        nc.tensor.matmul(
                            ps[:, bass.ts(ic, 512)],
                            wo_sb[:, bass.ts(i, 128)],
                            ot[:, i, bass.ts(ic, 512)],
                            start=(i == 0), stop=False)
                zparts[cc] = (wo_sb, ps)

            def z_finish(cc):
                wo_sb, ps = zparts.pop(cc)
                mm = NC - 1
                for ic in range(NIC):
                    nc.tensor.matmul(
                        ps[:, bass.ts(ic, 512)], wo_sb[:, bass.ts(mm, 128)],
                        ot[:, mm, bass.ts(ic, 512)], start=False, stop=True)
                zsb = zsbp.tile([128, T], F32, tag="zsb", name=f"zsb{cc}")
                nc.vector.tensor_copy(zsb[:], ps[:])
                nc.sync.dma_start(z_t[cc * 128:(cc + 1) * 128, :], zsb[:])

            normalize_pre(NC - 1)
            z_partial(0)               # PE-dense while the pre-chain is on DVE
            normalize_post(NC - 1)     # bc borrows the second S slot
            z_partial(1)
            for cc in range(NC):
                z_finish(cc)
                if cc + 2 < NC:
                    z_partial(cc + 2)
    nc.compile()
    return nc


def _group_w(wqkv, k):
    """Rows of Wqkv (3E, E) for q/k/v (k=0/1/2), grouped head-major.

    Row index layout: r = di*(3H) + k*H + h  ->  grouped[h*D+di, :].
    """
    w = np.asarray(wqkv, dtype=np.float32).reshape(D, 3, H, E)[:, k]   # [di, h, e]
    return np.ascontiguousarray(w.transpose(1, 0, 2).reshape(E, E))    # [h*D+di, e]


def _stream_layout(w_t):
    """Swizzle w^T [e*128+p, m*128+c] -> [m*128+p, e*128+c] so the device can
    stream output-block m as one contiguous [128, 1024] DMA."""
    a = np.asarray(w_t).reshape(NC, 128, NC, 128)
    return np.ascontiguousarray(a.transpose(2, 1, 0, 3).reshape(E, E))


def kernel(x, y, Wqkv1, Wqkv2, Wout1, Wout2):
    x = np.asarray(x, dtype=np.float32)
    y = np.asarray(y, dtype=np.float32)

    if "nc" not in _NC_CACHE:
        _NC_CACHE["nc"] = _build()
    nc = _NC_CACHE["nc"]

    # weight prep (host): grouped + transposed (f16 on-device dtype)
    wq1_t = np.ascontiguousarray(_group_w(Wqkv1, 0).T)
    wk1_t = np.ascontiguousarray(_group_w(Wqkv1, 1).T)
    wv1_t = np.ascontiguousarray(_group_w(Wqkv1, 2).T)
    wq2_t = np.ascontiguousarray(_group_w(Wqkv2, 0).T)
    wk2_t = np.ascontiguousarray(_group_w(Wqkv2, 1).T)
    wv2_t = np.ascontiguousarray(_group_w(Wqkv2, 2).T)
    wout1_t = np.ascontiguousarray(np.asarray(Wout1, dtype=np.float32).T)
    wout2_t = np.ascontiguousarray(np.asarray(Wout2, dtype=np.float32).T)

    in_maps = []
    for c in range(N_CORES):
        s, b = divmod(c, B)
        if s == 0:
            # stream-1 output: K,V from x via Wqkv1; Q from y via Wqkv2
            a_t, b_t = x[b].T, y[b].T
            wq, wk, wv, wo = wq2_t, wk1_t, wv1_t, wout1_t
        else:
            a_t, b_t = y[b].T, x[b].T
            wq, wk, wv, wo = wq1_t, wk2_t, wv2_t, wout2_t
        in_maps.append({
            "a_t": np.ascontiguousarray(a_t).astype(np.float16),
            "b_t": np.ascontiguousarray(b_t).astype(np.float16),
            "wq_t": _stream_layout(wq).astype(np.float16),
            "wk_t": _stream_layout(wk).astype(np.float16),
            "wv_t": wv.astype(np.float16),
            "wout_t": _stream_layout(wo).astype(np.float16),
        })

    trace = os.environ.get("BASS_KERNEL_TRACE", "0") == "1"
    if trace:
        try:
            from antenv.axon_hooks import get_axon_ntff_profile_hook  # noqa: F401
        except ImportError:
            trace = False
    ncores = int(os.environ.get("KCORES", str(N_CORES)))
    r = bass_utils.run_bass_kernel_spmd(nc, in_maps[:ncores], core_ids=list(range(ncores)),
                                        trace=trace)
    LAST_RESULTS["exec_time_ns"] = r.exec_time_ns
    LAST_RESULTS["profile_json"] = r.profile_json

    out1 = np.stack([r.results[b]["z_t"].T for b in range(B)]).astype(np.float32)
    out2 = np.stack([r.results[B + b]["z_t"].T for b in range(B)]).astype(np.float32)
    return out1, out2


# revision 37
# speedup vs baseline: 1.0019x; 1.0019x over previous
"""Trainium2 Bass kernel for nn_MultiHeadCrossAttention (B=4, T=1024, E=1024, H=16).

Sharding: 8 fully independent shards (output stream s, batch b), zero
cross-core communication. Stream-1 output xo@Wout1 needs K,V from x and Q
from y; stream-2 the reverse. Core c<4 computes stream-1 batch c; core c>=4
stream-2 batch c-4.

Per-core design: one flat software-pipelined stream over 64 (head-pair m,
key-chunk jc) units. ScalarE exp is the pacing target; all other work is
interleaved so the tensor engine keeps a dense backlog (HAM stays warm):
  unit (m, jc): S^T = K^T.T @ Q^T (row-paired K=64 MMs, [128,T] PSUM,
                one LDWEIGHTS per 2 MMs); P^T = exp(S^T/8) (2 ACTs);
                O' accumulates both ic halves per weight load
                ([128,512] 1-bank PSUM tiles, ones-row gives rowsums free).
  injections (relative to unit stream):
    m=0 units carry V-projection for chunk jc just-in-time (chunks 0-3 on
    the still-empty O' slots, 4-7 on the S pool).
    (m,0): normalize m-1: rowsum rows spread to partitions 0/32/64/96 so
           reciprocal_approx_fast runs 512-wide; K=1 ones-matmul broadcast;
           DVE mul into O^T f16. Frees the four O' accumulator banks.
    (m,1)/(m,2): Q^T/K^T projections for chunk m+1 into the freed O' banks
           (time-shared; weights streamed from HBM in host-swizzled
           contiguous blocks, one LDWEIGHTS per 2 MMs).
    (m,4): O' allocators + catch-up (deferred so the slot wait lands after
           proj released the banks; all waits resolve acyclically vs the
           in-order PE queue - anything a PE instruction waits on is
           produced by instructions emitted earlier).
  tail: normalize m=7, then Z^T = Wout^T.T @ O^T per 128-row chunk (m=7
  term accumulated last so Z overlaps the flush).
PSUM budget (8 banks): S units 2x[128,1024]=4, O'/proj/V time-shared
4x[128,512]=4; bc broadcasts borrow an S slot transiently.
Host pre-transposes/groups weights and activations; re-transposes outputs.
"""

import os
import sys

sys.path.insert(0, "/opt/trn_rl_repo")

import numpy as np
import ml_dtypes
from contextlib import ExitStack

import concourse.bass as bass
import concourse.mybir as mybir
import concourse.tile as tile
from concourse import bacc
from concourse import bass_utils

B, T, E, H = 4, 1024, 1024, 16
D = E // H            # 64
NC = E // 128         # 8 chunks of 128
NIC = T // 512        # 2 free-dim chunks of 512
N_CORES = 8

F32 = mybir.dt.float32
F16 = mybir.dt.float16
EXP = mybir.ActivationFunctionType.Exp

_NC_CACHE = {}
LAST_RESULTS = {}


def _build():
    nc = bacc.Bacc("TRN2", target_bir_lowering=False, debug=False,
                   enable_asserts=False, num_devices=N_CORES)
    a_t = nc.dram_tensor("a_t", (E, T), F16, kind="ExternalInput").ap()
    b_t = nc.dram_tensor("b_t", (E, T), F16, kind="ExternalInput").ap()
    wq_t = nc.dram_tensor("wq_t", (E, E), F16, kind="ExternalInput").ap()
    wk_t = nc.dram_tensor("wk_t", (E, E), F16, kind="ExternalInput").ap()
    wv_t = nc.dram_tensor("wv_t", (E, E), F16, kind="ExternalInput").ap()
    wout_t = nc.dram_tensor("wout_t", (E, E), F16, kind="ExternalInput").ap()
    z_t = nc.dram_tensor("z_t", (E, T), F32, kind="ExternalOutput").ap()

    # wq_t/wk_t/wout_t are host-swizzled: block m of w^T is the contiguous
    # [128, 1024] slice rows m*128..(m+1)*128, laid out [p, e*128+c].
    def wblock(w, m):
        return w[m * 128:(m + 1) * 128, :]

    with tile.TileContext(nc) as tc, ExitStack() as ctx:
        persist = ctx.enter_context(tc.tile_pool(name="persist", bufs=1))
        qt = persist.tile([128, NC, T], F16, tag="qt")
        kt = persist.tile([128, NC, T], F16, tag="kt")
        v = persist.tile([128, NC, H * (D + 1)], F16, tag="v")
        at_sb = persist.tile([128, NC, T], F16, tag="at")
        bt_sb = persist.tile([128, NC, T], F16, tag="bt")
        ot = persist.tile([128, NC, T], F16, tag="ot")      # normalized O^T
        ones_t = persist.tile([128, 128], F16, tag="ones")
        nc.vector.memset(ones_t[:], 1.0)
        # tiny dummy exp so the ~2.7us ACT_TABLE_LOAD happens during the
        # prologue DMA wait instead of right before the first real exp
        tbl = persist.tile([1, 16], F16, tag="tbl")
        nc.scalar.activation(tbl[:], ones_t[0:1, 0:16], EXP, scale=0.125)

        # DMA order matters: bt + first weight chunks first so Q0/K0 start
        # early; at/wv interleaved so V-proj can chase the transfers.
        wch = ctx.enter_context(tc.tile_pool(name="wch", bufs=2))
        wq0 = wch.tile([128, NC * 128], F16, tag="w", name="wq0")
        wk0 = wch.tile([128, NC * 128], F16, tag="w", name="wk0")
        for c in range(NC):
            nc.sync.dma_start(bt_sb[:, c, :], b_t[c * 128:(c + 1) * 128, :])
        nc.sync.dma_start(wq0[:], wblock(wq_t, 0))
        nc.sync.dma_start(wk0[:], wblock(wk_t, 0))
        for c in range(NC):
            nc.sync.dma_start(at_sb[:, c, :], a_t[c * 128:(c + 1) * 128, :])

        wvp = ctx.enter_context(tc.tile_pool(name="wv", bufs=1))
        wv_sb = wvp.tile([128, NC, E], F16, tag="wv")
        for c in range(NC):
            nc.sync.dma_start(wv_sb[:, c, :], wv_t[c * 128:(c + 1) * 128, :])

        # ones column per head in V (col D within each D+1 group) -> rowsums
        for m in range(NC):
            nc.vector.memset(
                v[:, m, :].rearrange("p (h x) -> p h x", x=D + 1)[:, :, D:D + 1], 1.0)

        ptp = ctx.enter_context(tc.tile_pool(name="pt", bufs=4))
        rsp = ctx.enter_context(tc.tile_pool(name="rsp", bufs=1))
        bcp = ctx.enter_context(tc.tile_pool(name="bcp", bufs=1))
        bigp = ctx.enter_context(tc.tile_pool(name="bigp", bufs=2, space="PSUM"))
        opool = ctx.enter_context(tc.tile_pool(name="op", bufs=4, space="PSUM"))

        def proj_chunk(w_sb, act_sb, out_sb, m, on_act, use_opool=False):
            """One [128, T] projection chunk (contract over e, weight block
            loaded once, both ic-half accumulators live)."""
            if use_opool:
                ps0 = opool.tile([128, 512], F32, tag="o", name=f"pj{m}a")
                ps1 = opool.tile([128, 512], F32, tag="o", name=f"pj{m}b")
            else:
                big = bigp.tile([128, T], F32, tag="big", name=f"pj{m}")
                ps0, ps1 = big[:, 0:512], big[:, 512:1024]
            for e in range(NC):
                st_e = dict(start=(e == 0), stop=(e == NC - 1))
                nc.tensor.matmul(ps0[:, :] if use_opool else ps0,
                                 w_sb[:, bass.ts(e, 128)],
                                 act_sb[:, e, 0:512], **st_e)
                nc.tensor.matmul(ps1[:, :] if use_opool else ps1,
                                 w_sb[:, bass.ts(e, 128)],
                                 act_sb[:, e, 512:1024], **st_e)
            for ic, ps in ((0, ps0), (1, ps1)):
                dst = out_sb[:, m, bass.ts(ic, 512)]
                if on_act:
                    nc.scalar.copy(dst, ps[:, :] if use_opool else ps)
                else:
                    nc.vector.tensor_copy(dst, ps[:, :] if use_opool else ps)

        def vproj_chunk(jc):
            """V chunk jc (natural layout), copy strided into (h, D+1).

            Chunks 0-3 use the (still empty) O'-accumulator slots so they
            don't contend with the S-unit double buffer."""
            if jc < 4:
                ps0 = opool.tile([128, 512], F32, tag="o", name=f"vp{jc}a")
                ps1 = opool.tile([128, 512], F32, tag="o", name=f"vp{jc}b")
                halves = (ps0, ps1)
            else:
                big = bigp.tile([128, T], F32, tag="big", name=f"vps{jc}")
                halves = (big[:, 0:512], big[:, 512:1024])
            for e in range(NC):
                for ic in range(NIC):
                    nc.tensor.matmul(
                        halves[ic], at_sb[:, e, bass.ts(jc, 128)],
                        wv_sb[:, e, bass.ts(ic, 512)],
                        start=(e == 0), stop=(e == NC - 1))
            for ic in range(NIC):
                nc.vector.tensor_copy(
                    v[:, jc, :].rearrange(
                        "p (h x) -> p h x", x=D + 1)[:, ic * 8:(ic + 1) * 8, 0:D],
                    halves[ic].rearrange("p (h x) -> p h x", x=D))

        # HAM warm-up: dense junk matmuls on the first bt chunk while the
        # remaining input DMAs stream, so Q0/K0 run at full clock
        warm = bigp.tile([128, T], F32, tag="big", name="warm")
        for _ in range(16):
            nc.tensor.matmul(warm[:, 0:512], bt_sb[:, 0, 0:128],
                             bt_sb[:, 0, 0:512], start=True, stop=True)

        proj_chunk(wq0, bt_sb, qt, 0, on_act=False)
        proj_chunk(wk0, at_sb, kt, 0, on_act=False)

        # per-m pipeline state
        state = {}

        NORM_ROWS = {(0, 0): 0, (0, 1): 32, (1, 0): 64, (1, 1): 96}

        def normalize_pre(m):
            """DVE-only prefix: rowsums -> 1/r (f16). The four rowsum rows
            are spread to partitions 0/32/64/96 so the reciprocal runs 512
            elements wide instead of 2048 serial."""
            st_ = state[m]
            rsf = rsp.tile([128, 512], F32, tag="rsf", name=f"rsf{m}")
            rsr = rsp.tile([128, 512], F32, tag="rsr", name=f"rsr{m}")
            rsr16 = rsp.tile([128, 512], F16, tag="rsr16", name=f"rsr16_{m}")
            for (h, ic), p in NORM_ROWS.items():
                nc.vector.tensor_copy(rsf[p:p + 1, :],
                                      st_[f"acc{ic}_{h}"][64:65, :])
            nc.vector.reciprocal_approx_fast(rsr[:], rsf[:])
            with nc.allow_low_precision(reason="1/r feeds f16 bc matmul"):
                nc.vector.tensor_copy(rsr16[:], rsr[:])
            st_["rsr16"] = rsr16

        def normalize_post(m):
            """Broadcast (K=1 ones-matmul) + DVE mul into O^T f16. Frees all
            four of m's O'-accumulator PSUM tiles; its bc matmuls precede
            any PE instruction waiting on those slots."""
            st_ = state[m]
            rsr16 = st_["rsr16"]
            bc = bigp.tile([128, T], F32, tag="big", name=f"bc{m}")
            for (h, ic), p in NORM_ROWS.items():
                nc.tensor.matmul(
                    bc[h * 64:(h + 1) * 64, bass.ts(ic, 512)],
                    ones_t[p:p + 1, 0:64], rsr16[p:p + 1, :],
                    start=True, stop=True, tile_position=(p, h * 64))
            bcs = bcp.tile([128, T], F32, tag="bcs", name=f"bcs{m}")
            nc.vector.tensor_copy(bcs[:], bc[:])
            with nc.allow_low_precision(reason="O^T f16 feeds f16 out-proj"):
                for ic in range(NIC):
                    s_ic = bass.ts(ic, 512)
                    nc.vector.tensor_mul(ot[0:64, m, s_ic],
                                         st_[f"acc{ic}_0"][0:64, :], bcs[0:64, s_ic])
                    nc.vector.tensor_mul(ot[64:128, m, s_ic],
                                         st_[f"acc{ic}_1"][0:64, :],
                                         bcs[64:128, s_ic])

        # ---------------- the 64-unit stream ----------------
        for u in range(NC * NC):
            m, jc = divmod(u, NC)
            hA, hB = 2 * m, 2 * m + 1
            if jc == 0:
                state[m] = {
                    "ptA": ptp.tile([128, NC, T], F16, tag="pt", name=f"ptA{m}"),
                    "ptB": ptp.tile([128, NC, T], F16, tag="pt", name=f"ptB{m}"),
                }
            st_ = state[m]
            ptA, ptB = st_["ptA"], st_["ptB"]

            ps_s = bigp.tile([128, T], F32, tag="big", name=f"sA{u}")
            ps_sB = bigp.tile([128, T], F32, tag="big", name=f"sB{u}")
            for ic in range(NIC):
                nc.tensor.matmul(
                    ps_s[:, bass.ts(ic, 512)],
                    kt[0:64, m, bass.ts(jc, 128)],
                    qt[0:64, m, bass.ts(ic, 512)],
                    start=True, stop=True)
            for ic in range(NIC):
                nc.tensor.matmul(
                    ps_sB[:, bass.ts(ic, 512)],
                    kt[64:128, m, bass.ts(jc, 128)],
                    qt[64:128, m, bass.ts(ic, 512)],
                    start=True, stop=True, tile_position=(64, 0))
            nc.scalar.activation(ptA[:, jc, :], ps_s[:], EXP, scale=0.125)
            nc.scalar.activation(ptB[:, jc, :], ps_sB[:], EXP, scale=0.125)

            if m == 0:
                vproj_chunk(jc)   # just-in-time V for O' below

            # ---- injections (pipelined work of m-1 / m+1); emitted BEFORE
            # this unit's O' matmuls so normalize's bc matmuls precede any
            # PE instruction that waits on the slots its muls release ----
            if jc == 0 and m >= 1:
                normalize_pre(m - 1)
                if m + 1 < NC:
                    # prefetch next chunk's weights so the proj matmuls at
                    # (m,2)/(m,3) never wait on DMA in the in-order PE queue
                    wq_sb = wch.tile([128, NC * 128], F16, tag="w",
                                     name=f"wq{m+1}")
                    wk_sb = wch.tile([128, NC * 128], F16, tag="w",
                                     name=f"wk{m+1}")
                    nc.sync.dma_start(wq_sb[:], wblock(wq_t, m + 1))
                    nc.sync.dma_start(wk_sb[:], wblock(wk_t, m + 1))
                    state[m]["wq"] = wq_sb
                    state[m]["wk"] = wk_sb
            elif jc == 1 and m >= 1:
                normalize_post(m - 1)
                del state[m - 1]
            elif jc == 2 and m >= 1 and m + 1 < NC:
                proj_chunk(state[m]["wq"], bt_sb, qt, m + 1, on_act=False,
                           use_opool=True)
            elif jc == 3 and m >= 1 and m + 1 < NC:
                proj_chunk(state[m]["wk"], at_sb, kt, m + 1, on_act=False,
                           use_opool=True)
            elif jc == 4 and m == 0:
                wq_sb = wch.tile([128, NC * 128], F16, tag="w", name="wq1")
                nc.sync.dma_start(wq_sb[:], wblock(wq_t, 1))
                proj_chunk(wq_sb, bt_sb, qt, 1, on_act=False)
            elif jc == 6 and m == 0:
                wk_sb = wch.tile([128, NC * 128], F16, tag="w", name="wk1")
                nc.sync.dma_start(wk_sb[:], wblock(wk_t, 1))
                proj_chunk(wk_sb, at_sb, kt, 1, on_act=False)

            # O' accumulation (both ic halves per weight load). Start
            # deferred to jc==4 so the slot wait lands after proj(m+1) has
            # released the time-shared accumulator slots.
            def o_mms(j2, first):
                stf = dict(start=first, stop=(j2 == NC - 1))
                for ic in range(NIC):
                    nc.tensor.matmul(st_[f"acc{ic}_0"][0:65, :],
                                     v[:, j2, bass.ts(hA, D + 1)],
                                     ptA[:, j2, bass.ts(ic, 512)], **stf)
                for ic in range(NIC):
                    nc.tensor.matmul(st_[f"acc{ic}_1"][0:65, :],
                                     v[:, j2, bass.ts(hB, D + 1)],
                                     ptB[:, j2, bass.ts(ic, 512)], **stf)

            if jc == 4:
                for ic in range(NIC):
                    for h, nm_ in ((0, "A"), (1, "B")):
                        st_[f"acc{ic}_{h}"] = opool.tile(
                            [128, 512], F32, tag="o", name=f"o{ic}{nm_}{m}")
                for j2 in range(3):
                    o_mms(j2, first=(j2 == 0))
            elif jc == 5:
                for j2 in (3, 4, 5):
                    o_mms(j2, first=False)
            elif jc > 5:
                o_mms(jc, first=False)

        # ---------------- flush m=7 + out-projection ----------------
        # Z chunk partials (m=0..6 terms) run while the m=7 normalize chain
        # is on DVE; each finish adds the m=7 term once ot[:,7] lands.
        with tc.tile_pool(name="woch", bufs=3) as wochp, \
             tc.tile_pool(name="zsb", bufs=2) as zsbp:
            zparts = {}
            wo_tiles = {}

            def wo_load(cc):
                wo_sb = wochp.tile([128, NC * 128], F16, tag="wo", name=f"wo{cc}")
                nc.sync.dma_start(wo_sb[:], wblock(wout_t, cc))
                wo_tiles[cc] = wo_sb

            def z_partial(cc):
                wo_sb = wo_tiles.pop(cc)
                ps = bigp.tile([128, T], F32, tag="big", name=f"z{cc}")
                for i in range(NC - 1):
                    for ic in range(NIC):
                        nc.tensor.matmul(
                            ps[:, bass.ts(ic, 512)],
                            wo_sb[:, bass.ts(i, 128)],
                            ot[:, i, bass.ts(ic, 512)],
                            start=(i == 0), stop=False)
                zparts[cc] = (wo_sb, ps)

            def z_finish(cc):
                wo_sb, ps = zparts.pop(cc)
                mm = NC - 1
                for ic in range(NIC):
                    nc.tensor.matmul(
                        ps[:, bass.ts(ic, 512)], wo_sb[:, bass.ts(mm, 128)],
                        ot[:, mm, bass.ts(ic, 512)], start=False, stop=True)
                zsb = zsbp.tile([128, T], F32, tag="zsb", name=f"zsb{cc}")
                nc.vector.tensor_copy(zsb[:], ps[:])
                nc.sync.dma_start(z_t[cc * 128:(cc + 1) * 128, :], zsb[:])

            wo_load(0)
            wo_load(1)
            normalize_pre(NC - 1)
            z_partial(0)               # PE-dense while the pre-chain is on DVE
            normalize_post(NC - 1)     # bc borrows the second S slot
            wo_load(2)
            z_partial(1)
            for cc in range(NC):
                z_finish(cc)
                if cc + 2 < NC:
                    z_partial(cc + 2)
                if cc + 3 < NC:
                    wo_load(cc + 3)
    nc.compile()
    return nc


def _group_w(wqkv, k):
    """Rows of Wqkv (3E, E) for q/k/v (k=0/1/2), grouped head-major.

    Row index layout: r = di*(3H) + k*H + h  ->  grouped[h*D+di, :].
    """
    w = np.asarray(wqkv, dtype=np.float32).reshape(D, 3, H, E)[:, k]   # [di, h, e]
    return np.ascontiguousarray(w.transpose(1, 0, 2).reshape(E, E))    # [h*D+di, e]


def _stream_layout(w_t):
    """Swizzle w^T [e*128+p, m*128+c] -> [m*128+p, e*128+c] so the device can
    stream output-block m as one contiguous [128, 1024] DMA."""
    a = np.asarray(w_t).reshape(NC, 128, NC, 128)
    return np.ascontiguousarray(a.transpose(2, 1, 0, 3).reshape(E, E))


def kernel(x, y, Wqkv1, Wqkv2, Wout1, Wout2):
    x = np.asarray(x, dtype=np.float32)
    y = np.asarray(y, dtype=np.float32)

    if "nc" not in _NC_CACHE:
        _NC_CACHE["nc"] = _build()
    nc = _NC_CACHE["nc"]

    # weight prep (host): grouped + transposed (f16 on-device dtype)
    wq1_t = np.ascontiguousarray(_group_w(Wqkv1, 0).T)
    wk1_t = np.ascontiguousarray(_group_w(Wqkv1, 1).T)
    wv1_t = np.ascontiguousarray(_group_w(Wqkv1, 2).T)
    wq2_t = np.ascontiguousarray(_group_w(Wqkv2, 0).T)
    wk2_t = np.ascontiguousarray(_group_w(Wqkv2, 1).T)
    wv2_t = np.ascontiguousarray(_group_w(Wqkv2, 2).T)
    wout1_t = np.ascontiguousarray(np.asarray(Wout1, dtype=np.float32).T)
    wout2_t = np.ascontiguousarray(np.asarray(Wout2, dtype=np.float32).T)

    in_maps = []
    for c in range(N_CORES):
        s, b = divmod(c, B)
        if s == 0:
            # stream-1 output: K,V from x via Wqkv1; Q from y via Wqkv2
            a_t, b_t = x[b].T, y[b].T
            wq, wk, wv, wo = wq2_t, wk1_t, wv1_t, wout1_t
        else:
            a_t, b_t = y[b].T, x[b].T
            wq, wk, wv, wo = wq1_t, wk2_t, wv2_t, wout2_t
        in_maps.append({
            "a_t": np.ascontiguousarray(a_t).astype(np.float16),
            "b_t": np.ascontiguousarray(b_t).astype(np.float16),
            "wq_t": _stream_layout(wq).astype(np.float16),
            "wk_t": _stream_layout(wk).astype(np.float16),
            "wv_t": wv.astype(np.float16),
            "wout_t": _stream_layout(wo).astype(np.float16),
        })

    trace = os.environ.get("BASS_KERNEL_TRACE", "0") == "1"
    if trace:
        try:
            from antenv.axon_hooks import get_axon_ntff_profile_hook  # noqa: F401
        except ImportError:
            trace = False
    ncores = int(os.environ.get("KCORES", str(N_CORES)))
    r = bass_utils.run_bass_kernel_spmd(nc, in_maps[:ncores], core_ids=list(range(ncores)),
                                        trace=trace)
    LAST_RESULTS["exec_time_ns"] = r.exec_time_ns
    LAST_RESULTS["profile_json"] = r.profile_json

    out1 = np.stack([r.results[b]["z_t"].T for b in range(B)]).astype(np.float32)
    out2 = np.stack([r.results[B + b]["z_t"].T for b in range(B)]).astype(np.float32)
    return out1, out2
